# revision 1
# baseline (speedup 1.0000x reference)
"""ContMix kernel for TRN2, 8 NeuronCores.

Sharding: (batch b, H-half) -> 8 cores. Each core computes out[b, :, r0:r0+28, :].

Pipeline (per core):
  pooling: f16 DVE+gpsimd split reduce -> ctx_p [C, 49]
  kf = (Wk/64) @ ctx_p ; G = Wq^T @ kf        (f16 matmuls)
  per 224-col chunk: logits lg = G^T x, expa = exp(lg)  (f16, no max-sub)
  per pair: dyn = expa^T @ [Wwd^T|1] -> d16 = dyn/denom   (PE + DVE, decoupled)
  per 2 pairs: gpsimd local_scatter -> M^T [112, 768]; PE transpose -> M chunks
  final: banded matmuls xt^T @ M accumulated over 3 chunks; out streamed in 4 DMAs
All DMA traffic f16; weights packed into one blob DMA.
"""

import numpy as np

B, C, H, W = 4, 384, 56, 56
KK, S = 5, 7
NCORES = 8
ROWS = H // 2              # 28 rows per core
NPIX = ROWS * W            # 1568
PADR = ROWS + 4            # 32 padded rows
PADW = 64                  # padded width
WSPACE = PADR * PADW       # 2048 padded pixels
NPAIR = ROWS // 2          # 14 output row-pairs
NCHUNK = PADR // 2         # 16 contraction chunks
D2 = C // 2                # 192
NI = 26                    # scatter idxs (25 taps + 1 denom col)
MCOLS = 3 * 128            # 384 = w''-space per pair
HW = H * W                 # 3136
FB = 1370                  # blob cols: wqA 0:384, wqB 384:768, wkt 768:1344, wwdt1 1344:1370

_cached = {}


def _build_nc():
    import concourse.tile as tile
    from concourse import bacc, mybir, library_config, masks

    f32, f16, i16 = mybir.dt.float32, mybir.dt.float16, mybir.dt.int16
    nc = bacc.Bacc("TRN2", target_bir_lowering=False, debug=False)

    blob_d = nc.dram_tensor("blob", [128, FB], f16, kind="ExternalInput")
    sidx_d = nc.dram_tensor("sidx", [2 * W, 2 * NI], i16, kind="ExternalInput")
    cx_d = nc.dram_tensor("cx", [C, HW], f16, kind="ExternalInput")
    xn_d = nc.dram_tensor("xn", [C, NPIX], f16, kind="ExternalInput")
    xt_d = nc.dram_tensor("xt", [WSPACE, C], f16, kind="ExternalInput")
    out_d = nc.dram_tensor("out", [C, NPIX], f16, kind="ExternalOutput")

    with tile.TileContext(nc) as tc:
        with (
            tc.tile_pool(name="big", bufs=1) as big,
            tc.tile_pool(name="wrk", bufs=3) as wrk,
            tc.tile_pool(name="mtp", bufs=7) as mtp,
            tc.tile_pool(name="ps", bufs=8, space="PSUM") as ps,
        ):
            # ---------------- input DMAs (SP queue = transfer order) --------------
            sidx_sb = big.tile([2 * W, 2 * NI], i16, tag="sidx")
            nc.sync.dma_start(out=sidx_sb[:], in_=sidx_d[:])
            cx_sb = big.tile([128, 3, HW], f16, tag="cx")
            for u in range(2):
                nc.sync.dma_start(out=cx_sb[:, u, :], in_=cx_d[u * 128:(u + 1) * 128, :])
            blob_sb = big.tile([128, FB], f16, tag="blob")
            nc.sync.dma_start(out=blob_sb[:], in_=blob_d[:])
            nc.sync.dma_start(out=cx_sb[:, 2, 0:24 * W], in_=cx_d[256:384, 0:24 * W])
            nc.sync.dma_start(out=cx_sb[:, 2, 24 * W:], in_=cx_d[256:384, 24 * W:])
            xn_sb = big.tile([128, 3, NPIX], f16, tag="xn")
            nc.sync.dma_start(out=xn_sb[:], in_=xn_d[:].rearrange("(u p) n -> p u n", p=128))
            xt_sb = big.tile([128, NCHUNK, C], f16, tag="xt")
            for hf in range(2):
                nc.sync.dma_start(
                    out=xt_sb[:, hf * 8:(hf + 1) * 8, :],
                    in_=xt_d[hf * 1024:(hf + 1) * 1024, :].rearrange("(t p) c -> p t c", p=128))

            nc.gpsimd.load_library(library_config.local_scatter)
            ident = big.tile([128, 128], f16, tag="ident")
            masks.make_identity(nc, ident[:])

            # ------- pooling (sum; /64 folded into wkt): DVE f16 add-tree ---------
            # rows first (i=8 within bin-row), then cols (j=8 within bin-col);
            # adds keep innermost packed so DVE 2x f16 mode applies.
            add = mybir.AluOpType.add
            ctx_p = big.tile([128, 3, S * S], f16, tag="ctxp")
            with nc.allow_low_precision(reason="f16 partial sums; |err|<<2e-2 gate"):
                for u, b0, b1_ in ((0, 0, S), (1, 0, S), (2, 0, 3), (2, 3, S)):
                    nb = b1_ - b0
                    v = cx_sb[:, u, b0 * 8 * W:b1_ * 8 * W].rearrange(
                        "p (bh i w) -> p bh i w", i=8, w=W)
                    a1 = wrk.tile([128, S, 4, W], f16, tag="a1")
                    nc.vector.tensor_tensor(out=a1[:, 0:nb], in0=v[:, :, 0:4, :],
                                            in1=v[:, :, 4:8, :], op=add)
                    a2 = wrk.tile([128, S, 2, W], f16, tag="a2")
                    nc.vector.tensor_tensor(out=a2[:, 0:nb], in0=a1[:, 0:nb, 0:2, :],
                                            in1=a1[:, 0:nb, 2:4, :], op=add)
                    a3 = wrk.tile([128, S, 1, W], f16, tag="a3")
                    nc.vector.tensor_tensor(out=a3[:, 0:nb], in0=a2[:, 0:nb, 0:1, :],
                                            in1=a2[:, 0:nb, 1:2, :], op=add)
                    d = a3[:, 0:nb].rearrange("p bh one (bw j) -> p (bh one) bw j", bw=S, j=8)
                    b1 = wrk.tile([128, S, S, 4], f16, tag="b1")
                    nc.vector.tensor_tensor(out=b1[:, 0:nb], in0=d[:, :, :, 0:4],
                                            in1=d[:, :, :, 4:8], op=add)
                    b2 = wrk.tile([128, S, S, 2], f16, tag="b2")
                    nc.vector.tensor_tensor(out=b2[:, 0:nb], in0=b1[:, 0:nb, :, 0:2],
                                            in1=b1[:, 0:nb, :, 2:4], op=add)
                    nc.vector.tensor_reduce(
                        out=ctx_p[:, u, b0 * S:b1_ * S],
                        in_=b2[:, 0:nb].rearrange("p bh bw j -> p (bh bw) j"),
                        axis=mybir.AxisListType.X, op=add)

            # ---------------- kf = (Wk/64) @ ctx_p : [192, 49] f16 ----------------
            kf_sb = [big.tile([128, S * S], f16, tag="kf0", name="kf0"),
                     big.tile([64, S * S], f16, tag="kf1", name="kf1")]
            for dc, dw in ((0, 128), (1, 64)):
                kf_ps = ps.tile([dw, S * S], f32, tag="ps")
                for u in range(3):
                    nc.tensor.matmul(kf_ps[:], blob_sb[:, 768 + u * 192 + dc * 128: 768 + u * 192 + dc * 128 + dw],
                                     ctx_p[:, u, :], start=(u == 0), stop=(u == 2))
                if dc == 0:
                    nc.vector.tensor_copy(kf_sb[dc][:], kf_ps[:])
                else:
                    nc.scalar.copy(kf_sb[dc][:], kf_ps[:])

            # ---------------- G = Wq^T @ kf : [384, 49] f16 ----------------
            g_sb = big.tile([128, 3, S * S], f16, tag="g")
            for u in range(3):
                g_ps = ps.tile([128, S * S], f32, tag="ps")
                nc.tensor.matmul(g_ps[:], blob_sb[:, u * 128:(u + 1) * 128], kf_sb[0][:],
                                 start=True, stop=False)
                nc.tensor.matmul(g_ps[:], blob_sb[0:64, 384 + u * 128:384 + (u + 1) * 128], kf_sb[1][:],
                                 start=False, stop=True)
                if u == 1:
                    nc.scalar.copy(g_sb[:, u, :], g_ps[:])
                else:
                    nc.vector.tensor_copy(g_sb[:, u, :], g_ps[:])

            # ------- B1: logits + exp + dyn + normalize (d16 for all pairs) -------
            expa = big.tile([S * S, NPIX], f16, tag="expa")
            d16a = big.tile([2 * W, NPAIR * NI], f16, tag="d16a")
            wwdt1 = blob_sb[0:S * S, 1344:1344 + NI]
            for ch in range(7):
                c0, c1 = ch * 224, (ch + 1) * 224
                lg = ps.tile([S * S, 224], f32, tag="ps")
                for u in range(3):
                    nc.tensor.matmul(lg[:], g_sb[:, u, :], xn_sb[:, u, c0:c1],
                                     start=(u == 0), stop=(u == 2))
                nc.scalar.activation(expa[:, c0:c1], lg[:],
                                     mybir.ActivationFunctionType.Exp)
                for p0 in (2 * ch, 2 * ch + 1):
                    dyn_ps = ps.tile([2 * W, NI], f32, tag="ps")
                    nc.tensor.matmul(dyn_ps[:], expa[:, p0 * 112:(p0 + 1) * 112],
                                     wwdt1, start=True, stop=True)
                    rec = wrk.tile([2 * W, 1], f32, tag="rec")
                    nc.vector.reciprocal(rec[:], dyn_ps[:, 25:26])
                    nc.vector.tensor_scalar_mul(d16a[:, p0 * NI:(p0 + 1) * NI], dyn_ps[:], rec[:])

            # ------- B2/B3: scatter (2 pairs/instr), PE transpose, final matmuls ---
            m_sb = big.tile([128, NPAIR * 3, 112], f16, tag="m")
            out_sb = big.tile([128, 3, NPIX], f16, tag="out")
            DMAS = {2: (0, 448), 4: (448, 896), 6: (896, 1344)}

            def finals(g):
                ga, gb = 2 * g, 2 * g + 2
                for cc in range(3):
                    po = ps.tile([128, 224], f32, tag="ps")
                    for pp in range(ga, gb):
                        for trel in range(3):
                            nc.tensor.matmul(po[:, (pp - ga) * 112:(pp - ga + 1) * 112],
                                             xt_sb[:, pp + trel, cc * 128:(cc + 1) * 128],
                                             m_sb[:, pp * 3 + trel, :],
                                             start=(trel == 0), stop=(trel == 2))
                    if cc == 1:
                        nc.vector.tensor_copy(out_sb[:, cc, ga * 112:gb * 112], po[:])
                    else:
                        nc.scalar.copy(out_sb[:, cc, ga * 112:gb * 112], po[:])

            for sp in range(7):
                mt = mtp.tile([2 * W, 2 * MCOLS], f16, tag="mt")
                nc.gpsimd.local_scatter(mt[:], d16a[:, sp * 2 * NI:(sp + 1) * 2 * NI],
                                        sidx_sb[:], channels=2 * W,
                                        num_elems=2 * MCOLS, num_idxs=2 * NI)
                tp_ps = ps.tile([128, 6, 112], f16, tag="ps")
                for t6 in range(6):
                    nc.tensor.transpose(tp_ps[:, t6, :], mt[:, t6 * 128:(t6 + 1) * 128],
                                        ident[0:112, 0:112])
                nc.vector.tensor_copy(m_sb[:, sp * 6:(sp + 1) * 6, :], tp_ps[:])

                # finals one scatter-pair behind: their M copy completed during
                # this sp's scatter, so PE never stalls on it.
                if sp >= 1:
                    finals(sp - 1)
                if sp in DMAS:
                    ca, cb = DMAS[sp]
                    nc.sync.dma_start(
                        out=out_d[:].rearrange("(u p) n -> p u n", p=128)[:, :, ca:cb],
                        in_=out_sb[:, :, ca:cb])
            finals(6)
            nc.sync.dma_start(
                out=out_d[:].rearrange("(u p) n -> p u n", p=128)[:, :, 1344:1568],
                in_=out_sb[:, :, 1344:1568])
    nc.finalize()
    return nc


def _static_inputs():
    # scatter index table for TWO adjacent pairs: pixel p = hl*56 + w,
    # tap j = 5*di + dj; second pair's M^T lives at col offset MCOLS.
    sidx = np.full((2 * W, 2 * NI), -1, np.int16)
    for half in range(2):
        for hl in range(2):
            for w in range(W):
                for di in range(KK):
                    for dj in range(KK):
                        sidx[hl * W + w, half * NI + 5 * di + dj] = \
                            half * MCOLS + (hl + di) * PADW + w + dj
    return sidx


def _prep(x, ctx, Wq, Wk, Wwd):
    sidx = _static_inputs()
    blob = np.zeros((128, FB), np.float16)
    blob[:, 0:384] = Wq[0:128, :]
    blob[0:64, 384:768] = Wq[128:192, :]
    wkt = (Wk.T / 64.0).astype(np.float16)          # [C, D2]
    for u in range(3):
        blob[:, 768 + u * 192:768 + (u + 1) * 192] = wkt[u * 128:(u + 1) * 128, :]
    blob[0:S * S, 1344:1344 + NI] = np.concatenate(
        [Wwd.T, np.ones((S * S, 1), np.float32)], axis=1)
    in_maps = []
    for core in range(NCORES):
        b, half = core // 2, core % 2
        r0 = half * ROWS
        xn = np.ascontiguousarray(x[b, :, r0:r0 + ROWS, :].reshape(C, NPIX)).astype(np.float16)
        xp = np.zeros((PADR, PADW, C), np.float32)
        lo, hi = max(0, r0 - 2), min(H, r0 + ROWS + 2)
        xp[lo - (r0 - 2):hi - (r0 - 2), 2:2 + W, :] = np.transpose(x[b, :, lo:hi, :], (1, 2, 0))
        xt = xp.reshape(WSPACE, C).astype(np.float16)
        cx = np.ascontiguousarray(ctx[b].reshape(C, HW)).astype(np.float16)
        in_maps.append(dict(blob=blob, sidx=sidx, cx=cx, xn=xn, xt=xt))
    return in_maps


def kernel(x, ctx, Wq, Wk, Wwd, _trace=False):
    from concourse.bass_utils import run_bass_kernel_spmd

    x, ctx = np.asarray(x), np.asarray(ctx)
    Wq, Wk, Wwd = np.asarray(Wq), np.asarray(Wk), np.asarray(Wwd)
    if "nc" not in _cached:
        _cached["nc"] = _build_nc()
    in_maps = _prep(x, ctx, Wq, Wk, Wwd)
    res = run_bass_kernel_spmd(_cached["nc"], in_maps, list(range(NCORES)), trace=_trace)
    _cached["last_result"] = res
    out = np.empty((B, C, H, W), np.float32)
    for core in range(NCORES):
        b, half = core // 2, core % 2
        r0 = half * ROWS
        out[b, :, r0:r0 + ROWS, :] = res.results[core]["out"].astype(np.float32).reshape(C, ROWS, W)
    return out



# revision 26
# speedup vs baseline: 1.1574x; 1.1574x over previous
"""ContMix kernel for TRN2, 8 NeuronCores — v3.

Sharding: (batch b, H-half) -> 8 cores. Each core computes out[b, :, r0:r0+28, :].

v3 pipeline (per core):
  PE warm-up matmuls (ident@ident) ramp the PE clock to full speed early.
  pooling on PE, c-major: lhsT = ctx_t chunk [112, 128c], rhs = ind [112, 7]
    -> po3 [128, u, br, bin] f32 accumulated 4 mms per (u, br); one DVE
    packing copy -> ctx_p [128, 3, 49]  (no transpose roundtrip)
  kf = Wk @ ctx_p ; G = Wq^T @ kf  (f16 matmuls)
  B1 pair-granular (112 cols): logits lg_p = G^T xn_p; exp; dyn_p = expa_p^T
    wwdt1; normalize -> d16a   (software-pipelined across engines)
  scatter calls of [1,2,2,2,2,2,2,1] pairs -> M^T; PE transpose -> M chunks
  finals: banded matmuls xt^T @ M per scatter-call group; out in 4 DMAs
DMA order: blob(+ind), sidx, ctx_t(8pc), xn(3pc), xt(4pc) — all f16.
"""

import numpy as np

B, C, H, W = 4, 384, 56, 56
KK, S = 5, 7
NCORES = 8
ROWS = H // 2              # 28 rows per core
NPIX = ROWS * W            # 1568
PADR = ROWS + 4            # 32 padded rows
PADW = 64                  # padded width
WSPACE = PADR * PADW       # 2048 padded pixels
NPAIR = ROWS // 2          # 14 output row-pairs
D2 = C // 2                # 192
NI = 26                    # scatter idxs (25 taps + 1 denom col)
MCOLS = 3 * 128            # 384 = w''-space per pair
HW = H * W                 # 3136
NPP = HW // 2              # 1568 pixel-pairs (full image, for pooling)
FB = 1377                  # blob: wqA 0:384, wqB 384:768, wkt 768:1344, wwdt1 1344:1370, ind 1370:1377
NWARM = 36                 # PE clock warm-up matmuls

# scatter-call pair grouping: starts early (1 pair), ends small (1 pair)
GROUPS = [(0, 1), (1, 3), (3, 5), (5, 7), (7, 9), (9, 11), (11, 13), (13, 14)]
# out DMA pieces fired at end of finals(g): g -> (col0, col1)
OUTDMAS = {2: (0, 560), 4: (560, 1008), 6: (1008, 1456), 7: (1456, 1568)}

_cached = {}


def _build_nc():
    import concourse.tile as tile
    from concourse import bacc, mybir, library_config, masks

    f32, f16, i16 = mybir.dt.float32, mybir.dt.float16, mybir.dt.int16
    nc = bacc.Bacc("TRN2", target_bir_lowering=False, debug=False)

    cxt_d = nc.dram_tensor("cxt", [NPP, 2 * C], f16, kind="ExternalInput")
    blob_d = nc.dram_tensor("blob", [128, FB], f16, kind="ExternalInput")
    xn_d = nc.dram_tensor("xn", [C, NPIX], f16, kind="ExternalInput")
    sidx_d = nc.dram_tensor("sidx", [2 * W, 2 * NI], i16, kind="ExternalInput")
    xt_d = nc.dram_tensor("xt", [WSPACE, C], f16, kind="ExternalInput")
    out_d = nc.dram_tensor("out", [C, NPIX], f16, kind="ExternalOutput")

    with tile.TileContext(nc) as tc:
        with (
            tc.tile_pool(name="big", bufs=1) as big,
            tc.tile_pool(name="wrk", bufs=3) as wrk,
            tc.tile_pool(name="mtp", bufs=3) as mtp,
            tc.tile_pool(name="ps", bufs=8, space="PSUM") as ps,
        ):
            # ---------------- input DMAs (SP queue = transfer order) --------------
            blob_sb = big.tile([128, FB], f16, tag="blob")
            nc.sync.dma_start(out=blob_sb[:], in_=blob_d[:])
            ind_sb = blob_sb[0:112, 1370:1377]
            sidx_sb = big.tile([2 * W, 2 * NI], i16, tag="sidx")
            nc.sync.dma_start(out=sidx_sb[:], in_=sidx_d[:])
            # ctx_t pieces: chunk t = 112 pixel-pairs = 4 image rows; bin-row br
            # = chunks 2br,2br+1.
            cxt_sb = big.tile([112, 14, 2 * C], f16, tag="cxt")
            CXP = [(0, 2), (2, 4), (4, 6), (6, 8), (8, 10), (10, 12), (12, 13), (13, 14)]
            for t0, t1 in CXP:
                nc.sync.dma_start(
                    out=cxt_sb[:, t0:t1, :],
                    in_=cxt_d[t0 * 112:t1 * 112, :].rearrange("(t p) c -> p t c", p=112))
            xn_sb = big.tile([128, 3, NPIX], f16, tag="xn")
            XNP = [(0, 560), (560, 1120), (1120, 1568)]
            for c0, c1 in XNP:
                nc.sync.dma_start(
                    out=xn_sb[:, :, c0:c1],
                    in_=xn_d[:].rearrange("(u p) n -> p u n", p=128)[:, :, c0:c1])
            xt_sb = big.tile([128, 16, C], f16, tag="xt")
            for q in range(4):
                nc.sync.dma_start(
                    out=xt_sb[:, q * 4:(q + 1) * 4, :],
                    in_=xt_d[q * 512:(q + 1) * 512, :].rearrange("(t p) c -> p t c", p=128))

            nc.gpsimd.load_library(library_config.local_scatter)
            ident = big.tile([128, 128], f16, tag="ident")
            masks.make_identity(nc, ident[:])

            # ---- PE warm-up: ramp the clock while ctx_t streams in ----------
            warm_ps = ps.tile([128, 128], f32, tag="ps", name="warm")
            for _ in range(NWARM):
                nc.tensor.matmul(warm_ps[:], ident[:], ident[:], start=True, stop=True)

            # ---- pooling on PE, c-major: po3u[u][c, br, bin] f32 -------------
            # One psum tile per u-chunk so accumulation groups never interleave
            # within a bank; group (u, br) = 4 consecutive mms.
            po3u = [ps.tile([128, S, 8], f32, tag="ps", name=f"po3{u}") for u in range(3)]
            for br in range(S):
                for u in range(3):
                    for k, (t, half) in enumerate(
                            ((2 * br, 0), (2 * br, 1), (2 * br + 1, 0), (2 * br + 1, 1))):
                        nc.tensor.matmul(
                            po3u[u][:, br, 0:S],
                            cxt_sb[:, t, half * C + u * 128: half * C + (u + 1) * 128],
                            ind_sb[:],
                            start=(k == 0), stop=(k == 3))
            ctxp = big.tile([128, 3, S, S], f16, tag="ctxp")
            for u in range(3):
                nc.vector.tensor_copy(ctxp[:, u], po3u[u][:, :, 0:S])

            # ---------------- kf = Wk @ ctx_p : [192, 49] f16 ----------------
            kf_sb = [big.tile([128, S * S], f16, tag="kf0", name="kf0"),
                     big.tile([64, S * S], f16, tag="kf1", name="kf1")]
            for dc, dw in ((0, 128), (1, 64)):
                kf_ps = ps.tile([dw, S * S], f32, tag="ps")
                for u in range(3):
                    nc.tensor.matmul(
                        kf_ps[:],
                        blob_sb[:, 768 + u * 192 + dc * 128: 768 + u * 192 + dc * 128 + dw],
                        ctxp[:, u].rearrange("p a b -> p (a b)"),
                        start=(u == 0), stop=(u == 2))
                if dc == 0:
                    nc.vector.tensor_copy(kf_sb[dc][:], kf_ps[:])
                else:
                    nc.scalar.copy(kf_sb[dc][:], kf_ps[:])

            # ---------------- G = Wq^T @ kf : [384, 49] f16 ----------------
            g_sb = big.tile([128, 3, S * S], f16, tag="g")
            for u in range(3):
                g_ps = ps.tile([128, S * S], f32, tag="ps")
                nc.tensor.matmul(g_ps[:], blob_sb[:, u * 128:(u + 1) * 128], kf_sb[0][:],
                                 start=True, stop=False)
                nc.tensor.matmul(g_ps[:], blob_sb[0:64, 384 + u * 128:384 + (u + 1) * 128],
                                 kf_sb[1][:], start=False, stop=True)
                if u == 1:
                    nc.scalar.copy(g_sb[:, u, :], g_ps[:])
                else:
                    nc.vector.tensor_copy(g_sb[:, u, :], g_ps[:])

            # ------- B1 (pair-granular) + scatter/transpose/finals pipeline -------
            expa = big.tile([S * S, NPAIR, 112], f16, tag="expa")
            d16a = big.tile([2 * W, NPAIR * NI], f16, tag="d16a")
            m_sb = big.tile([128, NPAIR * 3, 112], f16, tag="m")
            out_sb = big.tile([128, 3, NPIX], f16, tag="out")
            wwdt1 = blob_sb[0:S * S, 1344:1344 + NI]

            def logits(p):
                lg = ps.tile([S * S, 112], f32, tag="ps", name=f"lg{p}")
                for u in range(3):
                    nc.tensor.matmul(lg[:], g_sb[:, u, :],
                                     xn_sb[:, u, p * 112:(p + 1) * 112],
                                     start=(u == 0), stop=(u == 2))
                return lg

            def exp(p, lg):
                nc.scalar.activation(expa[:, p, :], lg[:],
                                     mybir.ActivationFunctionType.Exp)

            def dyn(p):
                dyn_ps = ps.tile([112, NI], f32, tag="ps", name=f"dy{p}")
                nc.tensor.matmul(dyn_ps[:], expa[:, p, :], wwdt1, start=True, stop=True)
                return dyn_ps

            def norm(p, dyn_ps):
                rec = wrk.tile([112, 1], f32, tag="rec")
                nc.vector.reciprocal(rec[:], dyn_ps[:, 25:26])
                nc.vector.tensor_scalar_mul(d16a[:, p * NI:(p + 1) * NI], dyn_ps[:], rec[:])

            def scatter(gi):
                p0, p1 = GROUPS[gi]
                npair = p1 - p0
                mt = mtp.tile([2 * W, 2 * MCOLS], f16, tag="mt", name=f"mt{gi}")
                nc.gpsimd.local_scatter(mt[:, 0:npair * MCOLS], d16a[:, p0 * NI:p1 * NI],
                                        sidx_sb[:, 0:npair * NI], channels=2 * W,
                                        num_elems=npair * MCOLS, num_idxs=npair * NI)
                return mt

            def transp(gi, mt):
                p0, p1 = GROUPS[gi]
                nch = (p1 - p0) * 3
                tp_ps = ps.tile([128, nch, 112], f16, tag="ps", name=f"tp{gi}")
                for t in range(nch):
                    nc.tensor.transpose(tp_ps[:, t, :], mt[:, t * 128:(t + 1) * 128],
                                        ident[0:112, 0:112])
                nc.vector.tensor_copy(m_sb[:, p0 * 3:p1 * 3, :], tp_ps[:])

            def finals(gi):
                p0, p1 = GROUPS[gi]
                w = (p1 - p0) * 112
                for cc in range(3):
                    po = ps.tile([128, w], f32, tag="ps", name=f"fin{gi}_{cc}")
                    for pp in range(p0, p1):
                        for trel in range(3):
                            nc.tensor.matmul(po[:, (pp - p0) * 112:(pp - p0 + 1) * 112],
                                             xt_sb[:, pp + trel, cc * 128:(cc + 1) * 128],
                                             m_sb[:, pp * 3 + trel, :],
                                             start=(trel == 0), stop=(trel == 2))
                    dst = out_sb[:, cc, p0 * 112:p1 * 112]
                    if cc == 1:
                        nc.vector.tensor_copy(dst, po[:])
                    else:
                        nc.scalar.copy(dst, po[:])
                if gi in OUTDMAS:
                    ca, cb = OUTDMAS[gi]
                    nc.sync.dma_start(
                        out=out_d[:].rearrange("(u p) n -> p u n", p=128)[:, :, ca:cb],
                        in_=out_sb[:, :, ca:cb])

            # Software pipeline; program order keeps each in-order engine queue
            # from parking behind a long-latency dependence.
            def LEDN(p):
                lg = logits(p)
                exp(p, lg)
                if p >= 2:
                    norm(p - 2, dyn(p - 2))

            LEDN(0), LEDN(1), LEDN(2)     # L0 E0, L1 E1, L2 E2 D0 N0
            mts = {0: scatter(0)}
            LEDN(3), LEDN(4)              # N1, N2
            mts[1] = scatter(1)
            transp(0, mts[0])
            LEDN(5), LEDN(6)              # N3, N4
            mts[2] = scatter(2)
            transp(1, mts[1])
            LEDN(7), LEDN(8)              # N5, N6
            mts[3] = scatter(3)
            finals(0)
            transp(2, mts[2])
            LEDN(9), LEDN(10)             # N7, N8
            mts[4] = scatter(4)
            finals(1)
            transp(3, mts[3])
            LEDN(11), LEDN(12)            # N9, N10
            mts[5] = scatter(5)
            finals(2)
            transp(4, mts[4])
            LEDN(13)                      # N11
            norm(12, dyn(12))
            mts[6] = scatter(6)
            finals(3)
            transp(5, mts[5])
            norm(13, dyn(13))
            mts[7] = scatter(7)
            finals(4)
            transp(6, mts[6])
            finals(5)
            transp(7, mts[7])
            finals(6)
            finals(7)
    nc.finalize()
    return nc


def _static_inputs():
    # scatter index table for TWO adjacent pairs: pixel p = hl*56 + w,
    # tap j = 5*di + dj; second pair's M^T lives at col offset MCOLS.
    sidx = np.full((2 * W, 2 * NI), -1, np.int16)
    for half in range(2):
        for hl in range(2):
            for w in range(W):
                for di in range(KK):
                    for dj in range(KK):
                        sidx[hl * W + w, half * NI + 5 * di + dj] = \
                            half * MCOLS + (hl + di) * PADW + w + dj
    # pooling indicator: partition i = pixel-pair (2i, 2i+1) within a 4-row
    # chunk; bin-col = (2i % 56)//8; value 1/64 (mean over the 8x8 bin).
    ind = np.zeros((112, S), np.float16)
    for i in range(112):
        ind[i, ((2 * i) % W) // 8] = 1.0 / 64.0
    return sidx, ind


def _prep(x, ctx, Wq, Wk, Wwd):
    sidx, ind = _static_inputs()
    blob = np.zeros((128, FB), np.float16)
    blob[:, 0:384] = Wq[0:128, :]
    blob[0:64, 384:768] = Wq[128:192, :]
    wkt = Wk.T.astype(np.float16)                   # [C, D2] (pooling ind is 1/64 mean)
    for u in range(3):
        blob[:, 768 + u * 192:768 + (u + 1) * 192] = wkt[u * 128:(u + 1) * 128, :]
    blob[0:S * S, 1344:1344 + NI] = np.concatenate(
        [Wwd.T, np.ones((S * S, 1), np.float32)], axis=1)
    blob[0:112, 1370:1377] = ind
    in_maps = []
    for core in range(NCORES):
        b, half = core // 2, core % 2
        r0 = half * ROWS
        xn = np.ascontiguousarray(x[b, :, r0:r0 + ROWS, :].reshape(C, NPIX)).astype(np.float16)
        xp = np.zeros((PADR, PADW, C), np.float32)
        lo, hi = max(0, r0 - 2), min(H, r0 + ROWS + 2)
        xp[lo - (r0 - 2):hi - (r0 - 2), 2:2 + W, :] = np.transpose(x[b, :, lo:hi, :], (1, 2, 0))
        xt = xp.reshape(WSPACE, C).astype(np.float16)
        cxt = np.ascontiguousarray(
            ctx[b].reshape(C, HW).T).astype(np.float16).reshape(NPP, 2 * C)
        in_maps.append(dict(cxt=cxt, blob=blob, xn=xn, sidx=sidx, xt=xt))
    return in_maps


def kernel(x, ctx, Wq, Wk, Wwd, _trace=False):
    from concourse.bass_utils import run_bass_kernel_spmd

    x, ctx = np.asarray(x), np.asarray(ctx)
    Wq, Wk, Wwd = np.asarray(Wq), np.asarray(Wk), np.asarray(Wwd)
    if "nc" not in _cached:
        _cached["nc"] = _build_nc()
    in_maps = _prep(x, ctx, Wq, Wk, Wwd)
    res = run_bass_kernel_spmd(_cached["nc"], in_maps, list(range(NCORES)), trace=_trace)
    _cached["last_result"] = res
    out = np.empty((B, C, H, W), np.float32)
    for core in range(NCORES):
        b, half = core // 2, core % 2
        r0 = half * ROWS
        out[b, :, r0:r0 + ROWS, :] = res.results[core]["out"].astype(np.float32).reshape(C, ROWS, W)
    return out


# revision 27
# speedup vs baseline: 1.1861x; 1.0247x over previous
"""ContMix kernel for TRN2, 8 NeuronCores — v3.

Sharding: (batch b, H-half) -> 8 cores. Each core computes out[b, :, r0:r0+28, :].

v3 pipeline (per core):
  PE warm-up matmuls (ident@ident) ramp the PE clock to full speed early.
  pooling on PE, c-major: lhsT = ctx_t chunk [112, 128c], rhs = ind [112, 7]
    -> po3 [128, u, br, bin] f32 accumulated 4 mms per (u, br); one DVE
    packing copy -> ctx_p [128, 3, 49]  (no transpose roundtrip)
  kf = Wk @ ctx_p ; G = Wq^T @ kf  (f16 matmuls)
  B1 pair-granular (112 cols): logits lg_p = G^T xn_p; exp; dyn_p = expa_p^T
    wwdt1; normalize -> d16a   (software-pipelined across engines)
  scatter calls of [1,2,2,2,2,2,2,1] pairs -> M^T; PE transpose -> M chunks
  finals: banded matmuls xt^T @ M per scatter-call group; out in 4 DMAs
DMA order: blob(+ind), sidx, ctx_t(8pc), xn(3pc), xt(4pc) — all f16.
"""

import numpy as np

B, C, H, W = 4, 384, 56, 56
KK, S = 5, 7
NCORES = 8
ROWS = H // 2              # 28 rows per core
NPIX = ROWS * W            # 1568
PADR = ROWS + 4            # 32 padded rows
PADW = 64                  # padded width
WSPACE = PADR * PADW       # 2048 padded pixels
NPAIR = ROWS // 2          # 14 output row-pairs
D2 = C // 2                # 192
NI = 26                    # scatter idxs (25 taps + 1 denom col)
MCOLS = 3 * 128            # 384 = w''-space per pair
HW = H * W                 # 3136
NPP = HW // 2              # 1568 pixel-pairs (full image, for pooling)
FB = 1569                  # blob: wqA 0:384, wqB 384:768, wkA 768:1152, wkB 1152:1536, wwdt1 1536:1562, ind 1562:1569
NWARM = 24                 # PE clock warm-up matmuls

# scatter-call pair grouping: starts early (1 pair), ends small (1 pair)
GROUPS = [(0, 1), (1, 3), (3, 5), (5, 7), (7, 9), (9, 11), (11, 13), (13, 14)]
# out DMA pieces fired at end of finals(g): g -> (col0, col1)
OUTDMAS = {2: (0, 560), 4: (560, 1008), 5: (1008, 1232), 6: (1232, 1456),
           7: (1456, 1568)}

_cached = {}


def _build_nc():
    import concourse.tile as tile
    from concourse import bacc, mybir, library_config, masks

    f32, f16, i16 = mybir.dt.float32, mybir.dt.float16, mybir.dt.int16
    nc = bacc.Bacc("TRN2", target_bir_lowering=False, debug=False)

    cxt_d = nc.dram_tensor("cxt", [NPP, 2 * C], f16, kind="ExternalInput")
    blob_d = nc.dram_tensor("blob", [128, FB], f16, kind="ExternalInput")
    xn_d = nc.dram_tensor("xn", [C, NPIX], f16, kind="ExternalInput")
    sidx_d = nc.dram_tensor("sidx", [2 * W, 2 * NI], i16, kind="ExternalInput")
    xt_d = nc.dram_tensor("xt", [WSPACE, C], f16, kind="ExternalInput")
    out_d = nc.dram_tensor("out", [C, NPIX], f16, kind="ExternalOutput")

    with tile.TileContext(nc) as tc:
        with (
            tc.tile_pool(name="big", bufs=1) as big,
            tc.tile_pool(name="wrk", bufs=3) as wrk,
            tc.tile_pool(name="mtp", bufs=3) as mtp,
            tc.tile_pool(name="ps", bufs=8, space="PSUM") as ps,
        ):
            # ---------------- input DMAs (SP queue = transfer order) --------------
            blob_sb = big.tile([128, FB], f16, tag="blob")
            nc.sync.dma_start(out=blob_sb[:], in_=blob_d[:])
            ind_sb = blob_sb[0:112, 1562:1569]
            sidx_sb = big.tile([2 * W, 2 * NI], i16, tag="sidx")
            nc.sync.dma_start(out=sidx_sb[:], in_=sidx_d[:])
            # ctx_t pieces: chunk t = 112 pixel-pairs = 4 image rows; bin-row br
            # = chunks 2br,2br+1.
            cxt_sb = big.tile([112, 14, 2 * C], f16, tag="cxt")
            CXP = [(0, 2), (2, 4), (4, 6), (6, 8), (8, 10), (10, 12), (12, 13), (13, 14)]
            for t0, t1 in CXP:
                nc.sync.dma_start(
                    out=cxt_sb[:, t0:t1, :],
                    in_=cxt_d[t0 * 112:t1 * 112, :].rearrange("(t p) c -> p t c", p=112))
            xn_sb = big.tile([128, 3, NPIX], f16, tag="xn")
            XNP = [(0, 560), (560, 1120), (1120, 1568)]
            for c0, c1 in XNP:
                nc.sync.dma_start(
                    out=xn_sb[:, :, c0:c1],
                    in_=xn_d[:].rearrange("(u p) n -> p u n", p=128)[:, :, c0:c1])
            xt_sb = big.tile([128, 16, C], f16, tag="xt")
            for q in range(4):
                nc.sync.dma_start(
                    out=xt_sb[:, q * 4:(q + 1) * 4, :],
                    in_=xt_d[q * 512:(q + 1) * 512, :].rearrange("(t p) c -> p t c", p=128))

            nc.gpsimd.load_library(library_config.local_scatter)
            ident = big.tile([128, 128], f16, tag="ident")
            masks.make_identity(nc, ident[:])

            # ---- PE warm-up: ramp the clock while ctx_t streams in ----------
            warm_ps = ps.tile([128, 128], f32, tag="ps", name="warm")
            for _ in range(NWARM):
                nc.tensor.matmul(warm_ps[:], ident[:], ident[:], start=True, stop=True)

            # ---- Tt = Wk^T @ Wq on device (warms PE with real work) ---------
            # Tt[c', c]; then G = sum_c' Tt[c', c] ctx_p[c', s] with no kf stage.
            tt_sb = big.tile([128, 3, C], f16, tag="tt")
            for cu in range(3):
                tt_ps = ps.tile([128, C], f32, tag="ps", name=f"tt{cu}")
                nc.tensor.matmul(tt_ps[:], blob_sb[:, 768 + cu * 128:768 + (cu + 1) * 128],
                                 blob_sb[:, 0:384], start=True, stop=False)
                nc.tensor.matmul(tt_ps[:], blob_sb[0:64, 1152 + cu * 128:1152 + (cu + 1) * 128],
                                 blob_sb[0:64, 384:768], start=False, stop=True)
                nc.vector.tensor_copy(tt_sb[:, cu, :], tt_ps[:])

            # ---- pooling on PE, c-major: po3u[u][c, br, bin] f32 -------------
            # One psum tile per u-chunk so accumulation groups never interleave
            # within a bank; group (u, br) = 4 consecutive mms.
            po3u = [ps.tile([128, S, 8], f32, tag="ps", name=f"po3{u}") for u in range(3)]
            for br in range(S):
                for u in range(3):
                    for k, (t, half) in enumerate(
                            ((2 * br, 0), (2 * br, 1), (2 * br + 1, 0), (2 * br + 1, 1))):
                        nc.tensor.matmul(
                            po3u[u][:, br, 0:S],
                            cxt_sb[:, t, half * C + u * 128: half * C + (u + 1) * 128],
                            ind_sb[:],
                            start=(k == 0), stop=(k == 3))
            ctxp = big.tile([128, 3, S, S], f16, tag="ctxp")
            for u in range(3):
                nc.vector.tensor_copy(ctxp[:, u], po3u[u][:, :, 0:S])

            # -------- G = Tt^T @ ctx_p : [384, 49] f16 (single stage) --------
            g_sb = big.tile([128, 3, S * S], f16, tag="g")
            for cu in range(3):
                g_ps = ps.tile([128, S * S], f32, tag="ps")
                for v in range(3):
                    nc.tensor.matmul(g_ps[:], tt_sb[:, v, cu * 128:(cu + 1) * 128],
                                     ctxp[:, v].rearrange("p a b -> p (a b)"),
                                     start=(v == 0), stop=(v == 2))
                if cu == 1:
                    nc.scalar.copy(g_sb[:, cu, :], g_ps[:])
                else:
                    nc.vector.tensor_copy(g_sb[:, cu, :], g_ps[:])

            # ------- B1 (pair-granular) + scatter/transpose/finals pipeline -------
            expa = big.tile([S * S, NPAIR, 112], f16, tag="expa")
            d16a = big.tile([2 * W, NPAIR * NI], f16, tag="d16a")
            m_sb = big.tile([128, NPAIR * 3, 112], f16, tag="m")
            out_sb = big.tile([128, 3, NPIX], f16, tag="out")
            wwdt1 = blob_sb[0:S * S, 1536:1536 + NI]

            def logits(p):
                lg = ps.tile([S * S, 112], f32, tag="ps", name=f"lg{p}")
                for u in range(3):
                    nc.tensor.matmul(lg[:], g_sb[:, u, :],
                                     xn_sb[:, u, p * 112:(p + 1) * 112],
                                     start=(u == 0), stop=(u == 2))
                return lg

            def exp(p, lg):
                nc.scalar.activation(expa[:, p, :], lg[:],
                                     mybir.ActivationFunctionType.Exp)

            def dyn(p):
                dyn_ps = ps.tile([112, NI], f32, tag="ps", name=f"dy{p}")
                nc.tensor.matmul(dyn_ps[:], expa[:, p, :], wwdt1, start=True, stop=True)
                return dyn_ps

            def norm(p, dyn_ps):
                rec = wrk.tile([112, 1], f32, tag="rec")
                nc.vector.reciprocal(rec[:], dyn_ps[:, 25:26])
                nc.vector.tensor_scalar_mul(d16a[:, p * NI:(p + 1) * NI], dyn_ps[:], rec[:])

            def scatter(gi):
                p0, p1 = GROUPS[gi]
                npair = p1 - p0
                mt = mtp.tile([2 * W, 2 * MCOLS], f16, tag="mt", name=f"mt{gi}")
                nc.gpsimd.local_scatter(mt[:, 0:npair * MCOLS], d16a[:, p0 * NI:p1 * NI],
                                        sidx_sb[:, 0:npair * NI], channels=2 * W,
                                        num_elems=npair * MCOLS, num_idxs=npair * NI)
                return mt

            def transp(gi, mt):
                p0, p1 = GROUPS[gi]
                nch = (p1 - p0) * 3
                tp_ps = ps.tile([128, nch, 112], f16, tag="ps", name=f"tp{gi}")
                for t in range(nch):
                    nc.tensor.transpose(tp_ps[:, t, :], mt[:, t * 128:(t + 1) * 128],
                                        ident[0:112, 0:112])
                nc.vector.tensor_copy(m_sb[:, p0 * 3:p1 * 3, :], tp_ps[:])

            def finals(gi):
                p0, p1 = GROUPS[gi]
                w = (p1 - p0) * 112
                for cc in range(3):
                    po = ps.tile([128, w], f32, tag="ps", name=f"fin{gi}_{cc}")
                    for pp in range(p0, p1):
                        for trel in range(3):
                            nc.tensor.matmul(po[:, (pp - p0) * 112:(pp - p0 + 1) * 112],
                                             xt_sb[:, pp + trel, cc * 128:(cc + 1) * 128],
                                             m_sb[:, pp * 3 + trel, :],
                                             start=(trel == 0), stop=(trel == 2))
                    dst = out_sb[:, cc, p0 * 112:p1 * 112]
                    if cc == 1:
                        nc.vector.tensor_copy(dst, po[:])
                    else:
                        nc.scalar.copy(dst, po[:])
                if gi in OUTDMAS:
                    ca, cb = OUTDMAS[gi]
                    nc.sync.dma_start(
                        out=out_d[:].rearrange("(u p) n -> p u n", p=128)[:, :, ca:cb],
                        in_=out_sb[:, :, ca:cb])

            # Software pipeline; program order keeps each in-order engine queue
            # from parking behind a long-latency dependence.
            def LEDN(p):
                lg = logits(p)
                exp(p, lg)
                if p >= 3:
                    norm(p - 3, dyn(p - 3))

            LEDN(0), LEDN(1), LEDN(2), LEDN(3)   # ... N0
            mts = {0: scatter(0)}
            LEDN(4), LEDN(5)              # N1, N2
            mts[1] = scatter(1)
            transp(0, mts[0])
            LEDN(6), LEDN(7)              # N3, N4
            mts[2] = scatter(2)
            transp(1, mts[1])
            LEDN(8), LEDN(9)              # N5, N6
            mts[3] = scatter(3)
            finals(0)
            transp(2, mts[2])
            LEDN(10), LEDN(11)            # N7, N8
            mts[4] = scatter(4)
            finals(1)
            transp(3, mts[3])
            LEDN(12), LEDN(13)            # N9, N10
            mts[5] = scatter(5)
            finals(2)
            transp(4, mts[4])
            norm(11, dyn(11))
            norm(12, dyn(12))
            mts[6] = scatter(6)
            finals(3)
            transp(5, mts[5])
            norm(13, dyn(13))
            mts[7] = scatter(7)
            finals(4)
            transp(6, mts[6])
            finals(5)
            transp(7, mts[7])
            finals(6)
            finals(7)
    nc.finalize()
    return nc


def _static_inputs():
    # scatter index table for TWO adjacent pairs: pixel p = hl*56 + w,
    # tap j = 5*di + dj; second pair's M^T lives at col offset MCOLS.
    sidx = np.full((2 * W, 2 * NI), -1, np.int16)
    for half in range(2):
        for hl in range(2):
            for w in range(W):
                for di in range(KK):
                    for dj in range(KK):
                        sidx[hl * W + w, half * NI + 5 * di + dj] = \
                            half * MCOLS + (hl + di) * PADW + w + dj
    # pooling indicator: partition i = pixel-pair (2i, 2i+1) within a 4-row
    # chunk; bin-col = (2i % 56)//8; value 1/64 (mean over the 8x8 bin).
    ind = np.zeros((112, S), np.float16)
    for i in range(112):
        ind[i, ((2 * i) % W) // 8] = 1.0 / 64.0
    return sidx, ind


def _prep(x, ctx, Wq, Wk, Wwd):
    sidx, ind = _static_inputs()
    blob = np.zeros((128, FB), np.float16)
    blob[:, 0:384] = Wq[0:128, :]
    blob[0:64, 384:768] = Wq[128:192, :]
    blob[:, 768:1152] = Wk[0:128, :]
    blob[0:64, 1152:1536] = Wk[128:192, :]
    blob[0:S * S, 1536:1536 + NI] = np.concatenate(
        [Wwd.T, np.ones((S * S, 1), np.float32)], axis=1)
    blob[0:112, 1562:1569] = ind
    in_maps = []
    for core in range(NCORES):
        b, half = core // 2, core % 2
        r0 = half * ROWS
        xn = np.ascontiguousarray(x[b, :, r0:r0 + ROWS, :].reshape(C, NPIX)).astype(np.float16)
        xp = np.zeros((PADR, PADW, C), np.float32)
        lo, hi = max(0, r0 - 2), min(H, r0 + ROWS + 2)
        xp[lo - (r0 - 2):hi - (r0 - 2), 2:2 + W, :] = np.transpose(x[b, :, lo:hi, :], (1, 2, 0))
        xt = xp.reshape(WSPACE, C).astype(np.float16)
        cxt = np.ascontiguousarray(
            ctx[b].reshape(C, HW).T).astype(np.float16).reshape(NPP, 2 * C)
        in_maps.append(dict(cxt=cxt, blob=blob, xn=xn, sidx=sidx, xt=xt))
    return in_maps


def kernel(x, ctx, Wq, Wk, Wwd, _trace=False):
    from concourse.bass_utils import run_bass_kernel_spmd

    x, ctx = np.asarray(x), np.asarray(ctx)
    Wq, Wk, Wwd = np.asarray(Wq), np.asarray(Wk), np.asarray(Wwd)
    if "nc" not in _cached:
        _cached["nc"] = _build_nc()
    in_maps = _prep(x, ctx, Wq, Wk, Wwd)
    res = run_bass_kernel_spmd(_cached["nc"], in_maps, list(range(NCORES)), trace=_trace)
    _cached["last_result"] = res
    out = np.empty((B, C, H, W), np.float32)
    for core in range(NCORES):
        b, half = core // 2, core % 2
        r0 = half * ROWS
        out[b, :, r0:r0 + ROWS, :] = res.results[core]["out"].astype(np.float32).reshape(C, ROWS, W)
    return out
